# revision 66
# baseline (speedup 1.0000x reference)
"""Trainium2 Bass kernel: 16-head RoPE attention block (B=4, T=2048, D=2048).

Sharding: tensor-parallel over heads. Each of the 8 cores owns 2 heads
(a 256-wide slice of the q/k/v projection output features) and computes a
full-width partial of the output projection; the host sums the 8 fp16
partials (the "all-reduce").

v3 structure (vs the riffled v2):
  - cross-block score pipeline: all score matmuls + EXPs flow through one
    global rate-limited queue, popped between every ~3 PE matmuls anywhere
    in the schedule. The scalar engine's EXP backlog builds during the
    projection/out-proj phases, so PV matmuls consume pre-built E tiles
    and never starve on the activation engine.
  - softmax denominators from a depth-3 bf16 add tree (8 pairs -> 4 quads
    -> 2 octs on the DVE) + two ones-matmuls per block (was 5).
  - V produced token-major directly by swapping matmul operands
    (stationary = x slice, moving = Wv) - no XBAR DMA transpose.
  - startup: ones via memset (no DRAM const), x chunk 0 / wk / wv split
    into pieces across the 4 DMA rings, ~shorter PE warm-up.
  - tail: the final out-projection block's DMA drains in 4 pieces across
    all 4 rings.
  - everything flows in bf16 (weights, x, q/k/v, exp(S), attention, Wo);
    PSUM accumulation stays fp32; outputs written as fp16 partials.
"""

import math

import numpy as np
import ml_dtypes

import concourse.bacc as bacc
import concourse.bass as bass
import concourse.mybir as mybir
import concourse.tile as tile
from concourse.bass_utils import run_bass_kernel_spmd

F32 = mybir.dt.float32
BF16 = mybir.dt.bfloat16
FP16 = mybir.dt.float16
EXP = mybir.ActivationFunctionType.Exp

# Problem shape (hardcoded; the harness calls kernel() with exactly these).
B = 4
T = 2048
D_MODEL = 2048
HEAD_DIM = 128
N_CORES = 8
ROPE_BASE = 10000.0

HPC = 2                      # heads per core
F_LOC = HPC * HEAD_DIM       # 256 local projection features per core
TCH = 512                    # stage-1 token chunk width
QCH = 512                    # stage-2 query chunk width
SCALE = 1.0 / math.sqrt(HEAD_DIM)

POP_SLOTS = 5.9              # PE 512-col-slots between score-pair pops
E_MAX = 12                   # outstanding (popped, not PV-consumed) E tiles
WARMUP = 215                 # PE warm-up matmuls (p-state ramp)


def build_module(b=B, t=T, d_model=D_MODEL, n_cores=N_CORES):
    """Build the per-core Bass module. All cores run the same program on
    different data (pure SPMD, no collectives)."""
    dt_ = d_model // 128     # 16 contraction tiles
    kt = t // 128            # 16 key tiles per batch
    cpb = t // TCH           # 4 stage-1 chunks per batch
    nqc = t // QCH           # 4 query chunks

    nc = bacc.Bacc(None, target_bir_lowering=False)

    # All DRAM tensors are host-packed partition-major so DMA descriptors
    # are 4-16KB contiguous runs (1KB descriptors leave the 16 DMA engines
    # descriptor-rate-bound at ~half throughput).
    ncg = b * t // TCH       # 16 global 512-token chunks
    xS = nc.dram_tensor("xS", [128, ncg, d_model // 128, TCH], BF16,
                        kind="ExternalInput")
    wqS = nc.dram_tensor("wqS", [128, d_model // 128, F_LOC], BF16,
                         kind="ExternalInput")
    wkS = nc.dram_tensor("wkS", [128, d_model // 128, F_LOC], BF16,
                         kind="ExternalInput")
    wvS = nc.dram_tensor("wvS", [128, d_model // 128, F_LOC], BF16,
                         kind="ExternalInput")
    woS = nc.dram_tensor("woS", [128, HPC, d_model], BF16,
                         kind="ExternalInput")
    cosT = nc.dram_tensor("cosT", [HEAD_DIM, t], BF16, kind="ExternalInput")
    rsinT = nc.dram_tensor("rsinT", [HEAD_DIM, t], BF16, kind="ExternalInput")
    outS = nc.dram_tensor("outS", [128, ncg, d_model // 128, TCH], FP16,
                          kind="ExternalOutput")

    with tile.TileContext(nc) as tc:
        with (
            tc.tile_pool(name="const", bufs=1) as constp,
            tc.tile_pool(name="wq", bufs=1) as wpool,
            tc.tile_pool(name="x", bufs=2) as xpool,
            tc.tile_pool(name="qkv", bufs=2) as qkvp,
            tc.tile_pool(name="t1", bufs=2) as tpool,
            tc.tile_pool(name="e", bufs=8) as epool,
            tc.tile_pool(name="tr", bufs=8) as trpool,
            tc.tile_pool(name="trf", bufs=6) as trfpool,
            tc.tile_pool(name="s2", bufs=2) as s2pool,
            tc.tile_pool(name="attn", bufs=2) as attnp,
            tc.tile_pool(name="s3o", bufs=5) as s3pool,
            tc.tile_pool(name="ps_a", bufs=2, space="PSUM") as ps_a,
            tc.tile_pool(name="ps_sc", bufs=2, space="PSUM") as ps_sc,
            tc.tile_pool(name="ps_pv", bufs=2, space="PSUM") as ps_pv,
        ):
            # ---- constants: ones from memset (no DRAM), so the PE
            # warm-up starts immediately ----
            ones_sb = constp.tile([128, 128], BF16)
            nc.vector.memset(ones_sb, 1.0)

            # PE warm-up: ramp the p-state while the weight/x DMAs land
            warm_ps = ps_a.tile([128, QCH], F32, tag="a", name="warm")
            for wu in range(WARMUP):
                nc.tensor.matmul(
                    warm_ps[:, 0:128], ones_sb, ones_sb, start=True, stop=True
                )

            # ---- initial loads: wk leads sync+scalar, x chunk 0 3-way
            # across the rings (sync/scalar HWDGE + gpsimd SWDGE), wq
            # behind x, wv on gpsimd. Steady-state x rides gpsimd only so
            # the scalar ring never blocks EXP dispatch. ----
            x_tiles = {}
            x_first = xpool.tile([128, dt_, TCH], BF16, name="x0_0", tag="x")

            w_sbs = []
            for wsrc_, wname in ((wqS, "wq"), (wkS, "wk"), (wvS, "wv")):
                wsb = wpool.tile([128, dt_, F_LOC], BF16, name=wname, tag=wname)
                w_sbs.append(wsb)
            # wk halves first (needed first)
            nc.sync.dma_start(out=w_sbs[1][:, 0:8, :], in_=wkS[:, 0:8, :])
            nc.scalar.dma_start(out=w_sbs[1][:, 8:16, :], in_=wkS[:, 8:16, :])
            # x chunk 0 split 3 ways by contraction blocks
            nc.sync.dma_start(out=x_first[:, 0:5, :], in_=xS[:, 0, 0:5, :])
            nc.scalar.dma_start(out=x_first[:, 5:10, :], in_=xS[:, 0, 5:10, :])
            nc.gpsimd.dma_start(out=x_first[:, 10:16, :], in_=xS[:, 0, 10:16, :])
            # chunk-0 order is k, v(ps_pv + vector copies), q - so wv
            # precedes wq, and cos/rsin (k-rope, emitted between v and q)
            # ride the scalar ring, which carries nothing else early
            nc.gpsimd.dma_start(out=w_sbs[2], in_=wvS[:, :, :])
            nc.sync.dma_start(out=w_sbs[0], in_=wqS[:, :, :])
            cos_sb = constp.tile([128, t], BF16)
            nc.scalar.dma_start(out=cos_sb, in_=cosT[:, :])
            rsin_sb = constp.tile([128, t], BF16)
            nc.scalar.dma_start(out=rsin_sb, in_=rsinT[:, :])
            # x chunk 1 split 3 ways behind the tables
            x_01 = xpool.tile([128, dt_, TCH], BF16, name="x0_1", tag="x")
            nc.sync.dma_start(out=x_01[:, 0:5, :], in_=xS[:, 1, 0:5, :])
            nc.scalar.dma_start(out=x_01[:, 5:10, :], in_=xS[:, 1, 5:10, :])
            nc.gpsimd.dma_start(out=x_01[:, 10:16, :], in_=xS[:, 1, 10:16, :])
            x_tiles[(0, 1)] = x_01
            wo_sb = wpool.tile([128, HPC, d_model], BF16, tag="wo")

            # per-batch double-buffered SBUF state, created lazily
            qk_sb = {}       # bi -> (q_sb, k_sb)  [128, HPC, t] bf16
            v_sb = {}        # bi -> [128, kt, HPC, 128] bf16
            attn_sb = {}     # bi -> [128, HPC, t] bf16

            # ============== global score pipeline ======================
            # Every (bi, h, qc, kti) score matmul + EXP flows through this
            # queue in block order. Pops are rate-limited to one per
            # ~POP_SLOTS 512-col PE slots (the EXP drain rate) and gated
            # on (a) the producing s1 chunks being emitted and (b) at most
            # E_MAX un-consumed E tiles outstanding.
            squeue = []
            for bi_ in range(b):
                for h_ in range(HPC):
                    for qc_ in range(nqc):
                        for kp_ in range(kt // 2):
                            squeue.append((bi_, h_, qc_, kp_))
            state = {"head": 0, "slots": 0.0, "e_out": 0, "pops_on": False}
            chunk_done = set()
            e_reg = {}       # (bi,h,qc) -> {kti: e_tile}
            e_cnt = {}       # (bi,h,qc) -> popped count
            tree = {}        # (bi,h,qc) -> dict(pr=[], qd=[], oct=[])
            dn_ps = {}       # (bi,h,qc) -> dn psum tile

            def tree_update(blk, e_pair):
                # pairs/quads on the DVE; octs + final on the otherwise
                # idle gpsimd engine -> a single ones-matmul denominator
                st = tree.setdefault(blk, {"pr": [], "qd": [], "oc": []})
                pr = trpool.tile([128, QCH], BF16, tag="tr", name="pr")
                nc.vector.tensor_add(pr, e_pair[:, 0, :], e_pair[:, 1, :])
                st["pr"].append(pr)
                if len(st["pr"]) % 2 == 0:
                    qd = trpool.tile([128, QCH], BF16, tag="tr", name="qd")
                    nc.vector.tensor_add(qd, st["pr"][-2], st["pr"][-1])
                    st["qd"].append(qd)
                    if len(st["qd"]) % 2 == 0:
                        oc = trfpool.tile([128, QCH], BF16, tag="trf", name="oc")
                        nc.vector.tensor_add(oc, st["qd"][-2], st["qd"][-1])
                        st["oc"].append(oc)
                        if len(st["oc"]) == 2:
                            fin = trfpool.tile(
                                [128, QCH], BF16, tag="trf", name="fin"
                            )
                            nc.vector.tensor_add(fin, st["oc"][0], st["oc"][1])
                            st["fin"] = fin

            def emit_dn(blk):
                """One ones-matmul over the full E sum; deferred until the
                PE is safely past the tree's cross-engine latency."""
                st = tree.pop(blk)
                dnp = ps_a.tile([128, QCH], F32, tag="a", name="dnp")
                nc.tensor.matmul(dnp, ones_sb, st["fin"], start=True, stop=True)
                dn_ps[blk] = dnp

            def pop_one(force=False):
                """Emit one score PAIR: two 512-col score matmuls into a
                bank-aligned [128,1024] psum, drained by a single wide EXP
                (the ~150ns fixed EXP overhead amortizes 2x)."""
                if state["head"] >= len(squeue):
                    return False
                bi_, h_, qc_, kp_ = squeue[state["head"]]
                if (bi_, max(qc_, (2 * kp_ + 1) // 4)) not in chunk_done:
                    return False
                if not force and (state["e_out"] >= E_MAX or not state["pops_on"]):
                    return False
                state["head"] += 1
                blk = (bi_, h_, qc_)
                q_t, k_t = qk_sb[bi_]
                sps = ps_sc.tile([128, 2, QCH], F32, tag="sc")
                for j in range(2):
                    kti_ = 2 * kp_ + j
                    nc.tensor.matmul(
                        sps[:, j, :],
                        k_t[:, h_, kti_ * 128 : (kti_ + 1) * 128],
                        q_t[:, h_, qc_ * QCH : (qc_ + 1) * QCH],
                        start=True,
                        stop=True,
                    )
                e_pair = epool.tile([128, 2, QCH], BF16, tag="E", name="e")
                nc.scalar.activation(e_pair, sps, EXP, scale=SCALE)
                e_reg.setdefault(blk, {})[kp_] = e_pair
                e_cnt[blk] = e_cnt.get(blk, 0) + 2
                state["e_out"] += 2
                tree_update(blk, e_pair)
                return True

            def maybe_pop(w):
                state["slots"] += w
                while state["slots"] >= POP_SLOTS and pop_one():
                    state["slots"] -= POP_SLOTS
                # when gated or drained, don't bank more than one pop
                state["slots"] = min(state["slots"], POP_SLOTS)

            # ================= emission units =========================

            def s1_load(bi, c):
                """Issue the x-chunk DMA (placed ahead of its compute).
                Steady state rides the gpsimd SWDGE ring only."""
                cg = bi * cpb + c
                x_sb = xpool.tile([128, dt_, TCH], BF16, name=f"x{bi}_{c}", tag="x")
                nc.gpsimd.dma_start(out=x_sb, in_=xS[:, cg, :, :])
                if (bi, c) == (0, 2):
                    # wo behind the first steady x chunk on gpsimd
                    nc.gpsimd.dma_start(out=wo_sb, in_=woS[:, :, :])
                x_tiles[(bi, c)] = x_sb

            def s1_chunk(bi, c):
                """Projections + rope + token-major V for 512 tokens."""
                if c == 0:
                    qk_sb[bi] = (
                        qkvp.tile([128, HPC, t], BF16, name=f"q{bi}", tag="q"),
                        qkvp.tile([128, HPC, t], BF16, name=f"k{bi}", tag="k"),
                    )
                    v_sb[bi] = qkvp.tile(
                        [128, kt, F_LOC], BF16, name=f"v{bi}", tag="v"
                    )
                off = c * TCH
                lsl = slice(off, off + TCH)
                x_sb = x_tiles.pop((bi, c))
                if (bi, c) == (0, 2):
                    # pops before here would sit in the PE queue gated on
                    # chunk-0/1 rope, which waits the cos/rsin DMAs
                    state["pops_on"] = True
                # chunk (0,0): consume x pieces in DMA-arrival order
                # (gpsimd piece lands first, then sync/scalar)
                if (bi, c) == (0, 0):
                    di_ord = list(range(10, dt_)) + list(range(10))
                else:
                    di_ord = list(range(dt_))

                def emit_rope(pi, ft, ps):
                    # rope: out = in*cos + rot_half(in)*sin
                    ro = tpool.tile([128, TCH], F32, tag="ro")
                    nc.vector.tensor_mul(ro, ps, cos_sb[:, lsl])
                    rt = tpool.tile([128, TCH], F32, tag="rt")
                    nc.vector.tensor_mul(rt[0:64], ps[64:128], rsin_sb[0:64, lsl])
                    nc.vector.tensor_mul(rt[64:128], ps[0:64], rsin_sb[64:128, lsl])
                    nc.vector.tensor_add(qk_sb[bi][pi][:, ft, lsl], ro, rt)

                def proj(pi, rope_now=True):
                    pss = []
                    for ft in range(HPC):
                        fsl = slice(ft * 128, (ft + 1) * 128)
                        ps = ps_a.tile([128, TCH], F32, tag="a", name="psqk")
                        for i_, di in enumerate(di_ord):
                            nc.tensor.matmul(
                                ps,
                                w_sbs[pi][:, di, fsl],
                                x_sb[:, di, :],
                                start=(i_ == 0),
                                stop=(i_ == dt_ - 1),
                            )
                            maybe_pop(1.0)
                        if rope_now:
                            emit_rope(pi, ft, ps)
                        pss.append(ps)
                    return pss

                def proj_v(ti, pool_, ptag):
                    ps = pool_.tile([128, TCH], F32, tag=ptag, name="psv")
                    t0_ = ti * 128
                    for i_, di in enumerate(di_ord):
                        nc.tensor.matmul(
                            ps[:, 0:F_LOC],
                            x_sb[:, di, t0_ : t0_ + 128],
                            w_sbs[2][:, di, :],
                            start=(i_ == 0),
                            stop=(i_ == dt_ - 1),
                        )
                        maybe_pop(F_LOC / 512.0)
                    return ps

                if (bi, c) == (0, 0):
                    # startup special: k (rope deferred), v on the idle
                    # ps_pv banks with vector copies (the scalar sequencer
                    # is still blocked issuing startup DMAs), k-rope, q.
                    k_ps = proj(1, rope_now=False)
                    for ti in range(TCH // 128):
                        ps = proj_v(ti, ps_pv, "pv")
                        nc.vector.tensor_copy(v_sb[bi][:, ti, :], ps[:, 0:F_LOC])
                    for ft, ps in enumerate(k_ps):
                        emit_rope(1, ft, ps)
                    proj(0)
                else:
                    # k first, q second, v last
                    proj(1)
                    proj(0)
                    for ti in range(TCH // 128):
                        ps = proj_v(ti, ps_a, "a")
                        j0 = (c * TCH + ti * 128) // 128
                        nc.scalar.copy(v_sb[bi][:, j0, :], ps[:, 0:F_LOC])
                chunk_done.add((bi, c))
                maybe_pop(0.0)

            def s2_block(bi, h, qc):
                """PV + denominator + normalize for one (batch, head,
                512-query chunk); E tiles come from the global pipeline."""
                if h == 0 and qc == 0:
                    attn_sb[bi] = attnp.tile(
                        [128, HPC, t], BF16, name=f"an{bi}", tag="an"
                    )
                blk = (bi, h, qc)
                pv = ps_pv.tile([128, QCH], F32, tag="pv")
                for kti in range(kt):
                    while e_cnt.get(blk, 0) < min(kt, kti + 5):
                        if not pop_one(force=True):
                            raise RuntimeError(f"score pipeline stuck at {blk}")
                    if blk not in dn_ps and kti >= 2 and e_cnt[blk] == kt:
                        emit_dn(blk)
                    nc.tensor.matmul(
                        pv,
                        v_sb[bi][:, kti, h * 128 : (h + 1) * 128],
                        e_reg[blk][kti // 2][:, kti % 2, :],
                        start=(kti == 0),
                        stop=(kti == kt - 1),
                    )
                    if kti % 2 == 1:
                        e_reg[blk].pop(kti // 2)
                    state["e_out"] -= 1
                    maybe_pop(1.0)
                e_reg.pop(blk, None)
                # dn_ps[blk] was emitted by the pipeline at this block's
                # 16th pop; reciprocal + normalize fuse into two DVE ops
                rec = s2pool.tile([128, QCH], F32, tag="rec")
                nc.vector.reciprocal_approx_fast(rec, dn_ps.pop(blk))
                nc.vector.tensor_mul(attn_sb[bi][:, h, qc * QCH : (qc + 1) * QCH], pv, rec)

            def s3_quarter(bi, c4, p4):
                """Out-projection partial for 4 of 16 output row-blocks of
                one 512-token chunk of batch bi; riffled finely so the
                psum->fp16 copies spread across the whole schedule."""
                off = c4 * TCH
                last = bi == b - 1 and c4 == cpb - 1
                osb = s3pool.tile([128, 4, TCH], FP16, tag="o", name="osb")
                for dj in range(4):
                    do = p4 * 4 + dj
                    pool_, ptag = (ps_a, "a") if do % 2 == 0 else (ps_pv, "pv")
                    ps = pool_.tile([128, TCH], F32, tag=ptag)
                    for ft in range(HPC):
                        nc.tensor.matmul(
                            ps,
                            wo_sb[:, ft, do * 128 : (do + 1) * 128],
                            attn_sb[bi][:, ft, off : off + TCH],
                            start=(ft == 0),
                            stop=(ft == HPC - 1),
                        )
                    # end region (no s1 work left) is scalar-bound: shift
                    # copies 1:3 scalar:vector there, else 2:2
                    sc_copy = (do % 2 == 0) if bi < b - 1 else (do % 4 == 0)
                    if sc_copy:
                        nc.scalar.copy(osb[:, dj, :], ps)
                    else:
                        nc.vector.tensor_copy(osb[:, dj, :], ps)
                    # pops AFTER the copy: a wide EXP queued ahead of the
                    # copy delays the psum-bank release and stalls the PE
                    maybe_pop(2.0)
                cg = bi * cpb + c4
                if last and p4 >= 2:
                    # final pieces drain as 2-do halves across the rings
                    r0, r1 = ((nc.gpsimd, nc.sync), (nc.scalar, nc.sync))[p4 - 2]
                    d0 = p4 * 4
                    r0.dma_start(out=outS[:, cg, d0 : d0 + 2, :], in_=osb[:, 0:2, :])
                    r1.dma_start(out=outS[:, cg, d0 + 2 : d0 + 4, :], in_=osb[:, 2:4, :])
                    return
                if last:
                    ring = (nc.sync, nc.scalar)[p4]
                else:
                    ring = nc.sync
                ring.dma_start(
                    out=outS[:, cg, p4 * 4 : (p4 + 1) * 4, :], in_=osb
                )

            # ================= riffled emission ========================
            x_tiles[(0, 0)] = x_first
            for c in range(cpb):
                s1_chunk(0, c)
                if c + 2 < cpb:
                    s1_load(0, c + 2)
            for bi in range(b):
                # s3 follows its own batch's s2 closely: s3q(bi, qc) runs
                # right after s2(bi,1,qc) (both heads done); qc=3 spills
                # into the next plan. s1 chunks of bi+1 interleave.
                plan = [
                    ("s1l", bi + 1, 0),
                    ("s2", bi, 0, 0), ("s3q", bi - 1, 3, 0),
                    ("s2", bi, 0, 1), ("s3q", bi - 1, 3, 1),
                    ("s3q", bi - 1, 3, 2),
                    ("s1l", bi + 1, 1), ("s3q", bi - 1, 3, 3),
                    ("s1", bi + 1, 0),
                    ("s2", bi, 0, 2), ("s2", bi, 0, 3),
                    ("s1l", bi + 1, 2), ("s1", bi + 1, 1),
                    ("s2", bi, 1, 0), ("s3q", bi, 0, 0), ("s3q", bi, 0, 1),
                    ("s2", bi, 1, 1), ("s3q", bi, 0, 2), ("s3q", bi, 0, 3),
                    ("s1l", bi + 1, 3), ("s1", bi + 1, 2),
                    ("s3q", bi, 1, 0), ("s2", bi, 1, 2),
                    ("s3q", bi, 1, 1), ("s3q", bi, 1, 2),
                    ("s2", bi, 1, 3), ("s3q", bi, 1, 3),
                    ("s1", bi + 1, 3),
                    ("s3q", bi, 2, 0), ("s3q", bi, 2, 1),
                    ("s3q", bi, 2, 2), ("s3q", bi, 2, 3),
                ]
                for unit in plan:
                    kind = unit[0]
                    if kind == "s1l" and unit[1] < b:
                        s1_load(unit[1], unit[2])
                    elif kind == "s1" and unit[1] < b:
                        s1_chunk(unit[1], unit[2])
                    elif kind == "s2":
                        s2_block(unit[1], unit[2], unit[3])
                    elif kind == "s3q" and unit[1] >= 0:
                        s3_quarter(unit[1], unit[2], unit[3])
            for p4_ in range(4):
                s3_quarter(b - 1, 3, p4_)

    nc.finalize()
    return nc


_module_cache = {}


def _get_module(b, t, d_model, n_cores):
    key = (b, t, d_model, n_cores)
    if key not in _module_cache:
        _module_cache[key] = build_module(b, t, d_model, n_cores)
    return _module_cache[key]


def _host_tables(t):
    half = HEAD_DIM // 2
    theta = 1.0 / (
        np.float32(ROPE_BASE)
        ** (np.arange(half, dtype=np.float32) / np.float32(half))
    )
    freqs = np.arange(t, dtype=np.float32)[:, None] * theta[None, :]
    emb = np.concatenate([freqs, freqs], axis=-1)  # (t, 128)
    bf16 = ml_dtypes.bfloat16
    cosT = np.ascontiguousarray(np.cos(emb).T.astype(bf16))
    sinT = np.sin(emb).T.astype(np.float32)
    rsinT = sinT.copy()
    rsinT[:half] = -sinT[:half]
    rsinT = np.ascontiguousarray(rsinT.astype(bf16))
    return cosT, rsinT


def _run(x, Wq, Wk, Wv, Wo, trace=False):
    b_, t_, d_ = x.shape
    n_cores = (d_ // HEAD_DIM) // HPC
    nc = _get_module(b_, t_, d_, n_cores)

    bf16 = ml_dtypes.bfloat16
    dt_ = d_ // 128
    ncg = b_ * t_ // TCH
    # partition-major packs (16KB x-lines, 8KB weight-lines)
    xT = x.reshape(b_ * t_, d_).T.astype(bf16)
    xS = np.ascontiguousarray(
        xT.reshape(dt_, 128, ncg, TCH).transpose(1, 2, 0, 3)
    )
    cosT, rsinT = _host_tables(t_)

    def packw(w_sl):  # [d_, F_LOC] (already transposed) -> [128, dt_, F_LOC]
        return np.ascontiguousarray(
            w_sl.reshape(dt_, 128, F_LOC).transpose(1, 0, 2)
        )

    in_maps = []
    for c in range(n_cores):
        fs = slice(c * F_LOC, (c + 1) * F_LOC)
        woT = Wo[:, fs].T.astype(bf16)  # [F_LOC, d_]
        in_maps.append(
            {
                "xS": xS,
                "wqS": packw(Wq[fs, :].T.astype(bf16)),
                "wkS": packw(Wk[fs, :].T.astype(bf16)),
                "wvS": packw(Wv[fs, :].T.astype(bf16)),
                "woS": np.ascontiguousarray(
                    woT.reshape(HPC, 128, d_).transpose(1, 0, 2)
                ),
                "cosT": cosT,
                "rsinT": rsinT,
            }
        )
    res = run_bass_kernel_spmd(
        nc, in_maps, core_ids=list(range(n_cores)), trace=trace
    )
    acc = res.results[0]["outS"].astype(np.float32)
    for c in range(1, n_cores):
        acc += res.results[c]["outS"].astype(np.float32)
    # unpack [128, ncg, dt_, TCH] -> [d_, b*t] -> (b, t, d_)
    full = acc.transpose(2, 0, 1, 3).reshape(d_, b_ * t_)
    out = np.ascontiguousarray(full.T).reshape(b_, t_, d_)
    return out, res


def kernel(x, Wq, Wk, Wv, Wo):
    x = np.asarray(x, dtype=np.float32)
    Wq = np.asarray(Wq, dtype=np.float32)
    Wk = np.asarray(Wk, dtype=np.float32)
    Wv = np.asarray(Wv, dtype=np.float32)
    Wo = np.asarray(Wo, dtype=np.float32)
    out, _ = _run(x, Wq, Wk, Wv, Wo, trace=False)
    return out


# revision 68
# speedup vs baseline: 1.0019x; 1.0019x over previous
"""Trainium2 Bass kernel: 16-head RoPE attention block (B=4, T=2048, D=2048).

Sharding: tensor-parallel over heads. Each of the 8 cores owns 2 heads
(a 256-wide slice of the q/k/v projection output features) and computes a
full-width partial of the output projection; the host sums the 8 fp16
partials (the "all-reduce").

v3 structure (vs the riffled v2):
  - cross-block score pipeline: all score matmuls + EXPs flow through one
    global rate-limited queue, popped between every ~3 PE matmuls anywhere
    in the schedule. The scalar engine's EXP backlog builds during the
    projection/out-proj phases, so PV matmuls consume pre-built E tiles
    and never starve on the activation engine.
  - softmax denominators from a depth-3 bf16 add tree (8 pairs -> 4 quads
    -> 2 octs on the DVE) + two ones-matmuls per block (was 5).
  - V produced token-major directly by swapping matmul operands
    (stationary = x slice, moving = Wv) - no XBAR DMA transpose.
  - startup: ones via memset (no DRAM const), x chunk 0 / wk / wv split
    into pieces across the 4 DMA rings, ~shorter PE warm-up.
  - tail: the final out-projection block's DMA drains in 4 pieces across
    all 4 rings.
  - everything flows in bf16 (weights, x, q/k/v, exp(S), attention, Wo);
    PSUM accumulation stays fp32; outputs written as fp16 partials.
"""

import math

import numpy as np
import ml_dtypes

import concourse.bacc as bacc
import concourse.bass as bass
import concourse.mybir as mybir
import concourse.tile as tile
from concourse.bass_utils import run_bass_kernel_spmd

F32 = mybir.dt.float32
BF16 = mybir.dt.bfloat16
FP16 = mybir.dt.float16
EXP = mybir.ActivationFunctionType.Exp

# Problem shape (hardcoded; the harness calls kernel() with exactly these).
B = 4
T = 2048
D_MODEL = 2048
HEAD_DIM = 128
N_CORES = 8
ROPE_BASE = 10000.0

HPC = 2                      # heads per core
F_LOC = HPC * HEAD_DIM       # 256 local projection features per core
TCH = 512                    # stage-1 token chunk width
QCH = 512                    # stage-2 query chunk width
SCALE = 1.0 / math.sqrt(HEAD_DIM)

POP_SLOTS = 5.6              # PE 512-col-slots between score-pair pops
E_MAX = 12                   # outstanding (popped, not PV-consumed) E tiles
WARMUP = 215                 # PE warm-up matmuls (p-state ramp)


def build_module(b=B, t=T, d_model=D_MODEL, n_cores=N_CORES):
    """Build the per-core Bass module. All cores run the same program on
    different data (pure SPMD, no collectives)."""
    dt_ = d_model // 128     # 16 contraction tiles
    kt = t // 128            # 16 key tiles per batch
    cpb = t // TCH           # 4 stage-1 chunks per batch
    nqc = t // QCH           # 4 query chunks

    nc = bacc.Bacc(None, target_bir_lowering=False)

    # All DRAM tensors are host-packed partition-major so DMA descriptors
    # are 4-16KB contiguous runs (1KB descriptors leave the 16 DMA engines
    # descriptor-rate-bound at ~half throughput).
    ncg = b * t // TCH       # 16 global 512-token chunks
    xS = nc.dram_tensor("xS", [128, ncg, d_model // 128, TCH], BF16,
                        kind="ExternalInput")
    wqS = nc.dram_tensor("wqS", [128, d_model // 128, F_LOC], BF16,
                         kind="ExternalInput")
    wkS = nc.dram_tensor("wkS", [128, d_model // 128, F_LOC], BF16,
                         kind="ExternalInput")
    wvS = nc.dram_tensor("wvS", [128, d_model // 128, F_LOC], BF16,
                         kind="ExternalInput")
    woS = nc.dram_tensor("woS", [128, HPC, d_model], BF16,
                         kind="ExternalInput")
    cosT = nc.dram_tensor("cosT", [HEAD_DIM, t], BF16, kind="ExternalInput")
    rsinT = nc.dram_tensor("rsinT", [HEAD_DIM, t], BF16, kind="ExternalInput")
    outS = nc.dram_tensor("outS", [128, ncg, d_model // 128, TCH], FP16,
                          kind="ExternalOutput")

    with tile.TileContext(nc) as tc:
        with (
            tc.tile_pool(name="const", bufs=1) as constp,
            tc.tile_pool(name="wq", bufs=1) as wpool,
            tc.tile_pool(name="x", bufs=2) as xpool,
            tc.tile_pool(name="qkv", bufs=2) as qkvp,
            tc.tile_pool(name="t1", bufs=2) as tpool,
            tc.tile_pool(name="e", bufs=8) as epool,
            tc.tile_pool(name="tr", bufs=8) as trpool,
            tc.tile_pool(name="trf", bufs=6) as trfpool,
            tc.tile_pool(name="s2", bufs=2) as s2pool,
            tc.tile_pool(name="attn", bufs=2) as attnp,
            tc.tile_pool(name="s3o", bufs=5) as s3pool,
            tc.tile_pool(name="ps_a", bufs=2, space="PSUM") as ps_a,
            tc.tile_pool(name="ps_sc", bufs=2, space="PSUM") as ps_sc,
            tc.tile_pool(name="ps_pv", bufs=2, space="PSUM") as ps_pv,
        ):
            # ---- constants: ones from memset (no DRAM), so the PE
            # warm-up starts immediately ----
            ones_sb = constp.tile([128, 128], BF16)
            nc.vector.memset(ones_sb, 1.0)

            # PE warm-up: ramp the p-state while the weight/x DMAs land
            warm_ps = ps_a.tile([128, QCH], F32, tag="a", name="warm")
            for wu in range(WARMUP):
                nc.tensor.matmul(
                    warm_ps[:, 0:128], ones_sb, ones_sb, start=True, stop=True
                )

            # ---- initial loads: wk leads sync+scalar, x chunk 0 3-way
            # across the rings (sync/scalar HWDGE + gpsimd SWDGE), wq
            # behind x, wv on gpsimd. Steady-state x rides gpsimd only so
            # the scalar ring never blocks EXP dispatch. ----
            x_tiles = {}
            x_first = xpool.tile([128, dt_, TCH], BF16, name="x0_0", tag="x")

            w_sbs = []
            for wsrc_, wname in ((wqS, "wq"), (wkS, "wk"), (wvS, "wv")):
                wsb = wpool.tile([128, dt_, F_LOC], BF16, name=wname, tag=wname)
                w_sbs.append(wsb)
            # wk halves first (needed first)
            nc.sync.dma_start(out=w_sbs[1][:, 0:8, :], in_=wkS[:, 0:8, :])
            nc.scalar.dma_start(out=w_sbs[1][:, 8:16, :], in_=wkS[:, 8:16, :])
            # x chunk 0 split 3 ways by contraction blocks
            nc.sync.dma_start(out=x_first[:, 0:5, :], in_=xS[:, 0, 0:5, :])
            nc.scalar.dma_start(out=x_first[:, 5:10, :], in_=xS[:, 0, 5:10, :])
            nc.gpsimd.dma_start(out=x_first[:, 10:16, :], in_=xS[:, 0, 10:16, :])
            # chunk-0 order is k, v(ps_pv + vector copies), q - so wv
            # precedes wq, and cos/rsin (k-rope, emitted between v and q)
            # ride the scalar ring, which carries nothing else early
            nc.gpsimd.dma_start(out=w_sbs[2], in_=wvS[:, :, :])
            nc.sync.dma_start(out=w_sbs[0], in_=wqS[:, :, :])
            cos_sb = constp.tile([128, t], BF16)
            nc.scalar.dma_start(out=cos_sb, in_=cosT[:, :])
            rsin_sb = constp.tile([128, t], BF16)
            nc.scalar.dma_start(out=rsin_sb, in_=rsinT[:, :])
            # x chunk 1 split 3 ways behind the tables
            x_01 = xpool.tile([128, dt_, TCH], BF16, name="x0_1", tag="x")
            nc.sync.dma_start(out=x_01[:, 0:5, :], in_=xS[:, 1, 0:5, :])
            nc.scalar.dma_start(out=x_01[:, 5:10, :], in_=xS[:, 1, 5:10, :])
            nc.gpsimd.dma_start(out=x_01[:, 10:16, :], in_=xS[:, 1, 10:16, :])
            x_tiles[(0, 1)] = x_01
            wo_sb = wpool.tile([128, HPC, d_model], BF16, tag="wo")

            # per-batch double-buffered SBUF state, created lazily
            qk_sb = {}       # bi -> (q_sb, k_sb)  [128, HPC, t] bf16
            v_sb = {}        # bi -> [128, kt, HPC, 128] bf16
            attn_sb = {}     # bi -> [128, HPC, t] bf16

            # ============== global score pipeline ======================
            # Every (bi, h, qc, kti) score matmul + EXP flows through this
            # queue in block order. Pops are rate-limited to one per
            # ~POP_SLOTS 512-col PE slots (the EXP drain rate) and gated
            # on (a) the producing s1 chunks being emitted and (b) at most
            # E_MAX un-consumed E tiles outstanding.
            squeue = []
            for bi_ in range(b):
                for h_ in range(HPC):
                    for qc_ in range(nqc):
                        for kp_ in range(kt // 2):
                            squeue.append((bi_, h_, qc_, kp_))
            state = {"head": 0, "slots": 0.0, "e_out": 0, "pops_on": False}
            chunk_done = set()
            e_reg = {}       # (bi,h,qc) -> {kti: e_tile}
            e_cnt = {}       # (bi,h,qc) -> popped count
            tree = {}        # (bi,h,qc) -> dict(pr=[], qd=[], oct=[])
            dn_ps = {}       # (bi,h,qc) -> dn psum tile

            def tree_update(blk, e_pair):
                # pairs/quads on the DVE; octs + final on the otherwise
                # idle gpsimd engine -> a single ones-matmul denominator
                st = tree.setdefault(blk, {"pr": [], "qd": [], "oc": []})
                pr = trpool.tile([128, QCH], BF16, tag="tr", name="pr")
                nc.vector.tensor_add(pr, e_pair[:, 0, :], e_pair[:, 1, :])
                st["pr"].append(pr)
                if len(st["pr"]) % 2 == 0:
                    qd = trpool.tile([128, QCH], BF16, tag="tr", name="qd")
                    nc.vector.tensor_add(qd, st["pr"][-2], st["pr"][-1])
                    st["qd"].append(qd)
                    if len(st["qd"]) % 2 == 0:
                        oc = trfpool.tile([128, QCH], BF16, tag="trf", name="oc")
                        nc.vector.tensor_add(oc, st["qd"][-2], st["qd"][-1])
                        st["oc"].append(oc)

            def emit_dn(blk):
                """Two ones-matmuls over the oct tiles; deferred until the
                PE is safely past the tree's cross-engine latency."""
                st = tree.pop(blk)
                dnp = ps_a.tile([128, QCH], F32, tag="a", name="dnp")
                nc.tensor.matmul(dnp, ones_sb, st["oc"][0], start=True, stop=False)
                nc.tensor.matmul(dnp, ones_sb, st["oc"][1], start=False, stop=True)
                dn_ps[blk] = dnp

            def pop_one(force=False):
                """Emit one score PAIR: two 512-col score matmuls into a
                bank-aligned [128,1024] psum, drained by a single wide EXP
                (the ~150ns fixed EXP overhead amortizes 2x)."""
                if state["head"] >= len(squeue):
                    return False
                bi_, h_, qc_, kp_ = squeue[state["head"]]
                if (bi_, max(qc_, (2 * kp_ + 1) // 4)) not in chunk_done:
                    return False
                if not force and (state["e_out"] >= E_MAX or not state["pops_on"]):
                    return False
                state["head"] += 1
                blk = (bi_, h_, qc_)
                q_t, k_t = qk_sb[bi_]
                sps = ps_sc.tile([128, 2, QCH], F32, tag="sc")
                for j in range(2):
                    kti_ = 2 * kp_ + j
                    nc.tensor.matmul(
                        sps[:, j, :],
                        k_t[:, h_, kti_ * 128 : (kti_ + 1) * 128],
                        q_t[:, h_, qc_ * QCH : (qc_ + 1) * QCH],
                        start=True,
                        stop=True,
                    )
                e_pair = epool.tile([128, 2, QCH], BF16, tag="E", name="e")
                nc.scalar.activation(e_pair, sps, EXP, scale=SCALE)
                e_reg.setdefault(blk, {})[kp_] = e_pair
                e_cnt[blk] = e_cnt.get(blk, 0) + 2
                state["e_out"] += 2
                tree_update(blk, e_pair)
                return True

            def maybe_pop(w):
                state["slots"] += w
                while state["slots"] >= POP_SLOTS and pop_one():
                    state["slots"] -= POP_SLOTS
                # when gated or drained, don't bank more than one pop
                state["slots"] = min(state["slots"], POP_SLOTS)

            # ================= emission units =========================

            def s1_load(bi, c):
                """Issue the x-chunk DMA (placed ahead of its compute).
                Steady state rides the gpsimd SWDGE ring only."""
                cg = bi * cpb + c
                x_sb = xpool.tile([128, dt_, TCH], BF16, name=f"x{bi}_{c}", tag="x")
                nc.gpsimd.dma_start(out=x_sb, in_=xS[:, cg, :, :])
                if (bi, c) == (0, 2):
                    # wo behind the first steady x chunk on gpsimd
                    nc.gpsimd.dma_start(out=wo_sb, in_=woS[:, :, :])
                x_tiles[(bi, c)] = x_sb

            def s1_chunk(bi, c):
                """Projections + rope + token-major V for 512 tokens."""
                if c == 0:
                    qk_sb[bi] = (
                        qkvp.tile([128, HPC, t], BF16, name=f"q{bi}", tag="q"),
                        qkvp.tile([128, HPC, t], BF16, name=f"k{bi}", tag="k"),
                    )
                    v_sb[bi] = qkvp.tile(
                        [128, kt, F_LOC], BF16, name=f"v{bi}", tag="v"
                    )
                off = c * TCH
                lsl = slice(off, off + TCH)
                x_sb = x_tiles.pop((bi, c))
                if (bi, c) == (0, 2):
                    # pops before here would sit in the PE queue gated on
                    # chunk-0/1 rope, which waits the cos/rsin DMAs
                    state["pops_on"] = True
                # chunk (0,0): consume x pieces in DMA-arrival order
                # (gpsimd piece lands first, then sync/scalar)
                if (bi, c) == (0, 0):
                    di_ord = list(range(10, dt_)) + list(range(10))
                else:
                    di_ord = list(range(dt_))

                def emit_rope(pi, ft, ps):
                    # rope: out = in*cos + rot_half(in)*sin
                    ro = tpool.tile([128, TCH], F32, tag="ro")
                    nc.vector.tensor_mul(ro, ps, cos_sb[:, lsl])
                    rt = tpool.tile([128, TCH], F32, tag="rt")
                    nc.vector.tensor_mul(rt[0:64], ps[64:128], rsin_sb[0:64, lsl])
                    nc.vector.tensor_mul(rt[64:128], ps[0:64], rsin_sb[64:128, lsl])
                    nc.vector.tensor_add(qk_sb[bi][pi][:, ft, lsl], ro, rt)

                def proj(pi, rope_now=True):
                    pss = []
                    for ft in range(HPC):
                        fsl = slice(ft * 128, (ft + 1) * 128)
                        ps = ps_a.tile([128, TCH], F32, tag="a", name="psqk")
                        for i_, di in enumerate(di_ord):
                            nc.tensor.matmul(
                                ps,
                                w_sbs[pi][:, di, fsl],
                                x_sb[:, di, :],
                                start=(i_ == 0),
                                stop=(i_ == dt_ - 1),
                            )
                            maybe_pop(1.0)
                        if rope_now:
                            emit_rope(pi, ft, ps)
                        pss.append(ps)
                    return pss

                def proj_v(ti, pool_, ptag):
                    ps = pool_.tile([128, TCH], F32, tag=ptag, name="psv")
                    t0_ = ti * 128
                    for i_, di in enumerate(di_ord):
                        nc.tensor.matmul(
                            ps[:, 0:F_LOC],
                            x_sb[:, di, t0_ : t0_ + 128],
                            w_sbs[2][:, di, :],
                            start=(i_ == 0),
                            stop=(i_ == dt_ - 1),
                        )
                        maybe_pop(F_LOC / 512.0)
                    return ps

                if (bi, c) == (0, 0):
                    # startup special: k (rope deferred), v on the idle
                    # ps_pv banks with vector copies (the scalar sequencer
                    # is still blocked issuing startup DMAs), k-rope, q.
                    k_ps = proj(1, rope_now=False)
                    for ti in range(TCH // 128):
                        ps = proj_v(ti, ps_pv, "pv")
                        nc.vector.tensor_copy(v_sb[bi][:, ti, :], ps[:, 0:F_LOC])
                    for ft, ps in enumerate(k_ps):
                        emit_rope(1, ft, ps)
                    proj(0)
                else:
                    # k first, q second, v last
                    proj(1)
                    proj(0)
                    for ti in range(TCH // 128):
                        ps = proj_v(ti, ps_a, "a")
                        j0 = (c * TCH + ti * 128) // 128
                        nc.scalar.copy(v_sb[bi][:, j0, :], ps[:, 0:F_LOC])
                chunk_done.add((bi, c))
                maybe_pop(0.0)

            def s2_block(bi, h, qc):
                """PV + denominator + normalize for one (batch, head,
                512-query chunk); E tiles come from the global pipeline."""
                if h == 0 and qc == 0:
                    attn_sb[bi] = attnp.tile(
                        [128, HPC, t], BF16, name=f"an{bi}", tag="an"
                    )
                blk = (bi, h, qc)
                pv = ps_pv.tile([128, QCH], F32, tag="pv")
                for kti in range(kt):
                    while e_cnt.get(blk, 0) < min(kt, kti + 5):
                        if not pop_one(force=True):
                            raise RuntimeError(f"score pipeline stuck at {blk}")
                    if blk not in dn_ps and kti >= 2 and e_cnt[blk] == kt:
                        emit_dn(blk)
                    nc.tensor.matmul(
                        pv,
                        v_sb[bi][:, kti, h * 128 : (h + 1) * 128],
                        e_reg[blk][kti // 2][:, kti % 2, :],
                        start=(kti == 0),
                        stop=(kti == kt - 1),
                    )
                    if kti % 2 == 1:
                        e_reg[blk].pop(kti // 2)
                    state["e_out"] -= 1
                    maybe_pop(1.0)
                e_reg.pop(blk, None)
                # dn_ps[blk] was emitted by the pipeline at this block's
                # 16th pop; reciprocal + normalize fuse into two DVE ops
                rec = s2pool.tile([128, QCH], F32, tag="rec")
                nc.vector.reciprocal_approx_fast(rec, dn_ps.pop(blk))
                nc.vector.tensor_mul(attn_sb[bi][:, h, qc * QCH : (qc + 1) * QCH], pv, rec)

            def s3_quarter(bi, c4, p4):
                """Out-projection partial for 4 of 16 output row-blocks of
                one 512-token chunk of batch bi; riffled finely so the
                psum->fp16 copies spread across the whole schedule."""
                off = c4 * TCH
                last = bi == b - 1 and c4 == cpb - 1
                osb = s3pool.tile([128, 4, TCH], FP16, tag="o", name="osb")
                for dj in range(4):
                    do = p4 * 4 + dj
                    pool_, ptag = (ps_a, "a") if do % 2 == 0 else (ps_pv, "pv")
                    ps = pool_.tile([128, TCH], F32, tag=ptag)
                    for ft in range(HPC):
                        nc.tensor.matmul(
                            ps,
                            wo_sb[:, ft, do * 128 : (do + 1) * 128],
                            attn_sb[bi][:, ft, off : off + TCH],
                            start=(ft == 0),
                            stop=(ft == HPC - 1),
                        )
                    # end region (no s1 work left) is scalar-bound: shift
                    # copies 1:3 scalar:vector there, else 2:2
                    sc_copy = (do % 2 == 0) if bi < b - 1 else (do % 4 == 0)
                    if sc_copy:
                        nc.scalar.copy(osb[:, dj, :], ps)
                    else:
                        nc.vector.tensor_copy(osb[:, dj, :], ps)
                    # pops AFTER the copy: a wide EXP queued ahead of the
                    # copy delays the psum-bank release and stalls the PE
                    maybe_pop(2.0)
                cg = bi * cpb + c4
                if last and p4 >= 2:
                    # final pieces drain as 2-do halves across the rings
                    r0, r1 = ((nc.gpsimd, nc.sync), (nc.scalar, nc.sync))[p4 - 2]
                    d0 = p4 * 4
                    r0.dma_start(out=outS[:, cg, d0 : d0 + 2, :], in_=osb[:, 0:2, :])
                    r1.dma_start(out=outS[:, cg, d0 + 2 : d0 + 4, :], in_=osb[:, 2:4, :])
                    return
                if last:
                    ring = (nc.sync, nc.scalar)[p4]
                else:
                    ring = nc.sync
                ring.dma_start(
                    out=outS[:, cg, p4 * 4 : (p4 + 1) * 4, :], in_=osb
                )

            # ================= riffled emission ========================
            x_tiles[(0, 0)] = x_first
            for c in range(cpb):
                s1_chunk(0, c)
                if c + 2 < cpb:
                    s1_load(0, c + 2)
            for bi in range(b):
                # s3 follows its own batch's s2 closely: s3q(bi, qc) runs
                # right after s2(bi,1,qc) (both heads done); qc=3 spills
                # into the next plan. s1 chunks of bi+1 interleave.
                plan = [
                    ("s1l", bi + 1, 0),
                    ("s2", bi, 0, 0), ("s3q", bi - 1, 3, 0),
                    ("s2", bi, 0, 1), ("s3q", bi - 1, 3, 1),
                    ("s3q", bi - 1, 3, 2),
                    ("s1l", bi + 1, 1), ("s3q", bi - 1, 3, 3),
                    ("s1", bi + 1, 0),
                    ("s2", bi, 0, 2), ("s2", bi, 0, 3),
                    ("s1l", bi + 1, 2), ("s1", bi + 1, 1),
                    ("s2", bi, 1, 0), ("s3q", bi, 0, 0), ("s3q", bi, 0, 1),
                    ("s2", bi, 1, 1), ("s3q", bi, 0, 2), ("s3q", bi, 0, 3),
                    ("s1l", bi + 1, 3), ("s1", bi + 1, 2),
                    ("s3q", bi, 1, 0), ("s2", bi, 1, 2),
                    ("s3q", bi, 1, 1), ("s3q", bi, 1, 2),
                    ("s2", bi, 1, 3), ("s3q", bi, 1, 3),
                    ("s1", bi + 1, 3),
                    ("s3q", bi, 2, 0), ("s3q", bi, 2, 1),
                    ("s3q", bi, 2, 2), ("s3q", bi, 2, 3),
                ]
                for unit in plan:
                    kind = unit[0]
                    if kind == "s1l" and unit[1] < b:
                        s1_load(unit[1], unit[2])
                    elif kind == "s1" and unit[1] < b:
                        s1_chunk(unit[1], unit[2])
                    elif kind == "s2":
                        s2_block(unit[1], unit[2], unit[3])
                    elif kind == "s3q" and unit[1] >= 0:
                        s3_quarter(unit[1], unit[2], unit[3])
            for p4_ in range(4):
                s3_quarter(b - 1, 3, p4_)

    nc.finalize()
    return nc


_module_cache = {}


def _get_module(b, t, d_model, n_cores):
    key = (b, t, d_model, n_cores)
    if key not in _module_cache:
        _module_cache[key] = build_module(b, t, d_model, n_cores)
    return _module_cache[key]


def _host_tables(t):
    half = HEAD_DIM // 2
    theta = 1.0 / (
        np.float32(ROPE_BASE)
        ** (np.arange(half, dtype=np.float32) / np.float32(half))
    )
    freqs = np.arange(t, dtype=np.float32)[:, None] * theta[None, :]
    emb = np.concatenate([freqs, freqs], axis=-1)  # (t, 128)
    bf16 = ml_dtypes.bfloat16
    cosT = np.ascontiguousarray(np.cos(emb).T.astype(bf16))
    sinT = np.sin(emb).T.astype(np.float32)
    rsinT = sinT.copy()
    rsinT[:half] = -sinT[:half]
    rsinT = np.ascontiguousarray(rsinT.astype(bf16))
    return cosT, rsinT


def _run(x, Wq, Wk, Wv, Wo, trace=False):
    b_, t_, d_ = x.shape
    n_cores = (d_ // HEAD_DIM) // HPC
    nc = _get_module(b_, t_, d_, n_cores)

    bf16 = ml_dtypes.bfloat16
    dt_ = d_ // 128
    ncg = b_ * t_ // TCH
    # partition-major packs (16KB x-lines, 8KB weight-lines)
    xT = x.reshape(b_ * t_, d_).T.astype(bf16)
    xS = np.ascontiguousarray(
        xT.reshape(dt_, 128, ncg, TCH).transpose(1, 2, 0, 3)
    )
    cosT, rsinT = _host_tables(t_)

    def packw(w_sl):  # [d_, F_LOC] (already transposed) -> [128, dt_, F_LOC]
        return np.ascontiguousarray(
            w_sl.reshape(dt_, 128, F_LOC).transpose(1, 0, 2)
        )

    in_maps = []
    for c in range(n_cores):
        fs = slice(c * F_LOC, (c + 1) * F_LOC)
        woT = Wo[:, fs].T.astype(bf16)  # [F_LOC, d_]
        in_maps.append(
            {
                "xS": xS,
                "wqS": packw(Wq[fs, :].T.astype(bf16)),
                "wkS": packw(Wk[fs, :].T.astype(bf16)),
                "wvS": packw(Wv[fs, :].T.astype(bf16)),
                "woS": np.ascontiguousarray(
                    woT.reshape(HPC, 128, d_).transpose(1, 0, 2)
                ),
                "cosT": cosT,
                "rsinT": rsinT,
            }
        )
    res = run_bass_kernel_spmd(
        nc, in_maps, core_ids=list(range(n_cores)), trace=trace
    )
    acc = res.results[0]["outS"].astype(np.float32)
    for c in range(1, n_cores):
        acc += res.results[c]["outS"].astype(np.float32)
    # unpack [128, ncg, dt_, TCH] -> [d_, b*t] -> (b, t, d_)
    full = acc.transpose(2, 0, 1, 3).reshape(d_, b_ * t_)
    out = np.ascontiguousarray(full.T).reshape(b_, t_, d_)
    return out, res


def kernel(x, Wq, Wk, Wv, Wo):
    x = np.asarray(x, dtype=np.float32)
    Wq = np.asarray(Wq, dtype=np.float32)
    Wk = np.asarray(Wk, dtype=np.float32)
    Wv = np.asarray(Wv, dtype=np.float32)
    Wo = np.asarray(Wo, dtype=np.float32)
    out, _ = _run(x, Wq, Wk, Wv, Wo, trace=False)
    return out
